# revision 28
# baseline (speedup 1.0000x reference)
"""CALoraLinear kernel for 8 TRN2 NeuronCores (Bass/Tile, SPMD).

Math (derived from the reference):
  orig = x @ W.T + bias
  top2 classes c1,c2 per row from pseudo_index[b, :64]
  g_j = <lora_A[c_j], x[b]>          (only rows 0..63 of lora_A are reachable)
  lora_out[b,o] = 16 * sum_c mask[b,c] * G[b,c] * lora_B[o,c]
  out = orig + lora_out + bias       (bias added twice)

Sharding: column-shard W across the 8 cores (each core owns 512 output
columns, full batch); x / lora_A / pseudo_index replicated. Host
concatenates the per-core [512, 512] blocks along the output axis.

Stream layout: one fp16 [x_k | w_k | a_k] block per k-tile (272 KB).
The first 4 k-tiles go as single-tile DMAs (fast PE start), the rest as
pairs, alternating the sync/scalar HWDGE rings. Small f32 inputs (ps,
psT, bS) ride the gpsimd SWDGE queue. Warm-up matmuls into a scratch
PSUM bank ramp the PE p-state during the DMA lead-in. The lora tail is
one DVE mult (PSUM-direct) + 4 matmuls interleaved with the last mains;
the output drains per batch-tile on both HWDGE rings.
"""

import os
import sys

for _p in ("/opt/trn_rl_repo",):
    if _p not in sys.path:
        sys.path.insert(0, _p)

import numpy as np

import concourse.bass as bass
import concourse.bacc as bacc
import concourse.mybir as mybir
from concourse.tile import TileContext, add_dep_helper
from concourse.bass_utils import run_bass_kernel_spmd


def _ensure_ntff_hook_module():
    """run_bass_kernel_spmd(trace=True) imports antenv.axon_hooks, which the
    agent image's antenv package lacks. Provide it (and register the real
    ctypes NTFF hook when available) so a tracing caller doesn't crash."""
    import types

    try:
        import antenv
    except ImportError:
        return
    if getattr(antenv, "axon_hooks", None) is not None:
        return
    mod = types.ModuleType("antenv.axon_hooks")
    state = {"hook": None}
    mod.set_axon_ntff_profile_hook = lambda h: state.__setitem__("hook", h)
    mod.get_axon_ntff_profile_hook = lambda: state["hook"]
    sys.modules["antenv.axon_hooks"] = mod
    antenv.axon_hooks = mod
    try:
        from trn_agent_boot.trn_boot import _ntff_profile_via_ctypes

        mod.set_axon_ntff_profile_hook(
            _ntff_profile_via_ctypes("/opt/axon/libaxon_pjrt.so")
        )
    except Exception:
        pass


_ensure_ntff_hook_module()

B, IN, OUT = 512, 4096, 4096
NUM_CLASS, RANK = 64, 8
NCORES = 8
OUT_L = OUT // NCORES  # 512
P = 128
KT = IN // P           # 32 k-tiles
BT = B // P            # 4 batch tiles

# per-k-tile column layout in the stream: [x: B][w: OUT_L][a: 64]
KW = B + OUT_L + NUM_CLASS  # 1088
XO, WO, AO = 0, B, B + OUT_L

# DMA chunk schedule: (start_k, n_k). Singles first (fast PE start), then
# pairs. Chunks alternate sync/scalar; GP_CHUNKS (by start_k) go to the
# gpsimd SWDGE queue instead as a third bandwidth source.
NSINGLE = int(os.environ.get("NSINGLE", "32"))
_sched = [(k, 1) for k in range(NSINGLE)]
k = NSINGLE
while k < KT:
    n = min(2, KT - k)
    _sched.append((k, n))
    k += n
CHUNKS = _sched
GP_CHUNKS = {
    int(v) for v in os.environ.get("GP_CHUNKS", "10,18,26").split(",") if v
}

WARM = int(os.environ.get("WARM", "28"))  # warm-up matmuls (128 rows each)

F32 = mybir.dt.float32
F32R = mybir.dt.float32r
F16 = mybir.dt.float16
X = mybir.AxisListType.X

_cache = {}
# test.py reads this after a traced run for HW exec time
last_results = None


def _build():
    key = "nc"
    if key in _cache:
        return _cache[key]
    nc = bacc.Bacc(
        bass.get_trn_type() or "TRN2",
        target_bir_lowering=False,
        debug=False,
        num_devices=NCORES,
    )

    # flat stream buffer; each chunk is host-packed as a [P, n*KW] block
    xw = nc.dram_tensor("xw", [KT * P * KW], F16, kind="ExternalInput")
    ps_d = nc.dram_tensor("ps", [P, BT * NUM_CLASS], F32, kind="ExternalInput")
    psT_d = nc.dram_tensor("psT", [NUM_CLASS, B], F32, kind="ExternalInput")
    bS_d = nc.dram_tensor("bS", [NUM_CLASS + 1, OUT_L], F32R, kind="ExternalInput")
    # fp16 output (host upcasts): halves the output drain at ~2.8e-4 rel err
    out = nc.dram_tensor("out", [B, OUT_L], F16, kind="ExternalOutput")

    with TileContext(nc) as tc:
        with (
            tc.tile_pool(name="xwp", bufs=1) as xwpool,
            tc.tile_pool(name="sml", bufs=1) as spool,
            tc.tile_pool(name="tl", bufs=1) as tpool,
            tc.tile_pool(name="op", bufs=1) as opool,
            tc.tile_pool(name="dr", bufs=1, space="DRAM") as dpool,
            tc.tile_pool(name="ps", bufs=1, space="PSUM") as ppool,
        ):
            # ---- resident stream buffer; chunk DMAs on both HWDGE rings
            # plus the gpsimd SWDGE queue for a few mid-stream chunks ----
            # (+64 pad cols so the last G lhsT [ak | pad] stays in bounds)
            xw_sb = xwpool.tile([P, KT * KW + NUM_CLASS], F16)
            nc.vector.memset(xw_sb[:, KT * KW : KT * KW + NUM_CLASS], 0.0)
            dma_done = {}
            hw_i = 0
            for k0, n in CHUNKS:
                if k0 in GP_CHUNKS:
                    dma_eng = nc.gpsimd
                else:
                    dma_eng = nc.sync if hw_i % 2 == 0 else nc.scalar
                    hw_i += 1
                dst = xw_sb[:, k0 * KW : (k0 + n) * KW]
                src = xw[k0 * P * KW : (k0 + n) * P * KW].rearrange(
                    "(p d) -> p d", p=P
                )
                dma_done[k0] = dma_eng.dma_start(out=dst, in_=src)

            # ---- small inputs on the gpsimd SWDGE queue, behind the gp
            # stream chunks and gated so their transfers don't compete with
            # the early stream (they are not needed until the lora tail) ----
            ps_sb = spool.tile([P, BT * NUM_CLASS], F32)
            ps_dma = nc.gpsimd.dma_start(out=ps_sb, in_=ps_d[:, :])
            psT_sb = spool.tile([NUM_CLASS, B], F32)
            nc.gpsimd.dma_start(out=psT_sb, in_=psT_d[:, :])
            bS_sb = spool.tile([NUM_CLASS + 1, OUT_L], F32R)
            nc.gpsimd.dma_start(out=bS_sb, in_=bS_d[:, :])
            add_dep_helper(
                ps_dma.ins, dma_done[8].ins,
                reason="small inputs yield early DMA bandwidth to the stream",
            )

            # ---- PE warm-up: ramp the p-state during the DMA lead-in ----
            warm_src = spool.tile([P, P], F16)
            nc.vector.memset(warm_src, 0.0)
            warm_ps = ppool.tile([P, P], F32, tag="warm", name="warm_ps")
            for w in range(WARM):
                nc.tensor.matmul(
                    warm_ps, lhsT=warm_src, rhs=warm_src, start=True, stop=True
                )

            # ---- top-2 threshold per batch row (DVE, alongside matmuls) ----
            m2col = spool.tile([P, BT], F32)
            for bt in range(BT):
                pt = ps_sb[:, bt * NUM_CLASS : (bt + 1) * NUM_CLASS]
                m1 = spool.tile([P, 1], F32, tag=f"m1_{bt}")
                nc.vector.reduce_max(out=m1, in_=pt, axis=X)
                negmask = spool.tile([P, NUM_CLASS], F32, tag=f"nm_{bt}")
                # (pt >= m1) * -1e30  -> additive mask that kills the max
                nc.vector.tensor_scalar(
                    out=negmask,
                    in0=pt,
                    scalar1=m1,
                    scalar2=-1.0e30,
                    op0=mybir.AluOpType.is_ge,
                    op1=mybir.AluOpType.mult,
                )
                p2 = spool.tile([P, NUM_CLASS], F32, tag=f"p2_{bt}")
                nc.vector.tensor_tensor(
                    out=p2, in0=pt, in1=negmask, op=mybir.AluOpType.add
                )
                nc.vector.reduce_max(out=m2col[:, bt : bt + 1], in_=p2, axis=X)

            # threshold shuffle on the GPSIMD (SWDGE) path: partition->free
            # [128, BT] -> flat [B] via a DRAM bounce, then broadcast-read
            # across 64 partitions (step-0 source dim).
            m2d = dpool.tile([BT, P], F32)
            nc.gpsimd.dma_start(out=m2d.rearrange("bt p -> p bt"), in_=m2col[:, :])
            thr_sb = spool.tile([NUM_CLASS, B], F32)
            nc.gpsimd.dma_start(
                out=thr_sb,
                in_=m2d.rearrange("bt p -> (bt p)")[None, :].broadcast_to(
                    [NUM_CLASS, B]
                ),
            )
            # maskT = (psT >= thr): mid-stream, plain SBUF DVE op
            maskT = tpool.tile([NUM_CLASS, B], F32)
            nc.vector.tensor_tensor(
                out=maskT, in0=psT_sb, in1=thr_sb, op=mybir.AluOpType.is_ge
            )
            # ht rows: 0..63 filled at the end; row 64 = ones (written early)
            ht = tpool.tile([NUM_CLASS + 1, B], F32R)
            nc.vector.tensor_scalar(
                out=ht[NUM_CLASS : NUM_CLASS + 1, :],
                in0=psT_sb[0:1, :],
                scalar1=0.0,
                scalar2=1.0,
                op0=mybir.AluOpType.mult,
                op1=mybir.AluOpType.add,
            )

            # ---- PSUM accumulators ----
            mps = [
                ppool.tile([P, OUT_L], F32, tag=f"main{bt}", name=f"main{bt}")
                for bt in range(BT)
            ]
            # G uses a full 128-partition accumulator: its lhsT is padded to
            # 128 columns so every matmul keeps PE tile_size (128,128) — a
            # (128,64) G matmul forces an array reconfig costing ~215ns/k.
            # Rows 64:128 accumulate garbage and are never read.
            gt_ps = ppool.tile([P, B], F32, tag="gt", name="gt_ps")

            def xk(k):
                return xw_sb[:, k * KW + XO : k * KW + XO + B]

            def wk(k):
                return xw_sb[:, k * KW + WO : k * KW + WO + OUT_L]

            def ak_pad(k):
                # [ak | next 64 cols] -> [128, 128] stationary operand
                return xw_sb[:, k * KW + AO : k * KW + AO + 2 * NUM_CLASS]

            def main_mm(k, bt):
                nc.tensor.matmul(
                    mps[bt],
                    lhsT=xk(k)[:, bt * P : (bt + 1) * P],
                    rhs=wk(k),
                    start=(k == 0),
                    stop=False,
                )

            def g_mm(k):
                nc.tensor.matmul(
                    gt_ps,
                    lhsT=ak_pad(k),
                    rhs=xk(k),
                    start=(k == 0),
                    stop=(k == KT - 1),
                )

            # main stream: mains then G per k; last two k run G first so the
            # DVE lora chain overlaps the final main matmuls
            for k in range(KT - 2):
                for bt in range(BT):
                    main_mm(k, bt)
                g_mm(k)
            g_mm(KT - 2)
            g_mm(KT - 1)
            for bt in range(BT):
                main_mm(KT - 2, bt)

            # ht[0:64] = G * maskT (PSUM-direct read on in0)
            nc.vector.tensor_tensor(
                out=ht[0:NUM_CLASS, :], in0=gt_ps[0:NUM_CLASS, :], in1=maskT,
                op=mybir.AluOpType.mult,
            )

            # ---- finale: per bt: last main k, lora tail matmul, copy, DMA out
            o_all = opool.tile([P, BT * OUT_L], F16)
            dma_eng = [nc.sync, nc.scalar, nc.gpsimd, nc.scalar]
            for bt in range(BT):
                main_mm(KT - 1, bt)
            for bt in range(BT):
                nc.tensor.matmul(
                    mps[bt],
                    lhsT=ht[:, bt * P : (bt + 1) * P],
                    rhs=bS_sb,
                    start=False,
                    stop=True,
                )
                osl = o_all[:, bt * OUT_L : (bt + 1) * OUT_L]
                if bt % 2 == 0:
                    nc.vector.tensor_copy(out=osl, in_=mps[bt])
                else:
                    nc.scalar.activation(
                        out=osl, in_=mps[bt],
                        func=mybir.ActivationFunctionType.Copy,
                    )
                dma_eng[bt].dma_start(out=out[bt * P : (bt + 1) * P, :], in_=osl)

    nc.finalize()
    _cache[key] = nc
    return nc


def _pack_inputs(x, pseudo_index, weight, bias, lora_A, lora_B):
    """Build the interleaved per-core xw stream + replicated small inputs."""
    xT = np.ascontiguousarray(x.T)                   # [IN, B]
    aT = np.ascontiguousarray(lora_A[:NUM_CLASS].T)  # [IN, 64]
    x3 = xT.reshape(KT, P, B)
    a3 = aT.reshape(KT, P, NUM_CLASS)

    ps = np.ascontiguousarray(
        pseudo_index.reshape(BT, P, NUM_CLASS)
        .transpose(1, 0, 2)
        .reshape(P, BT * NUM_CLASS)
    )
    psT = np.ascontiguousarray(pseudo_index.T)

    in_maps = []
    for i in range(NCORES):
        o0 = i * OUT_L
        wTi = weight[o0 : o0 + OUT_L].T              # [IN, OUT_L] (view)
        w3 = wTi.reshape(KT, P, OUT_L)
        k3 = np.empty((KT, P, KW), dtype=np.float16)
        k3[:, :, XO : XO + B] = x3
        k3[:, :, WO : WO + OUT_L] = w3
        k3[:, :, AO : AO + NUM_CLASS] = a3
        # pack per the chunk schedule: each chunk becomes a [P, n*KW] block
        xwi = np.empty(KT * P * KW, dtype=np.float16)
        for k0, n in CHUNKS:
            xwi[k0 * P * KW : (k0 + n) * P * KW] = (
                k3[k0 : k0 + n].transpose(1, 0, 2).reshape(-1)
            )
        bS = np.empty((NUM_CLASS + 1, OUT_L), dtype=np.float32)
        bS[:NUM_CLASS] = 16.0 * lora_B[o0 : o0 + OUT_L, :NUM_CLASS].T
        bS[NUM_CLASS] = 2.0 * bias[o0 : o0 + OUT_L]
        in_maps.append({"xw": xwi, "ps": ps, "psT": psT, "bS": bS})
    return in_maps


def kernel(x, pseudo_index, weight, bias, lora_A, lora_B):
    global last_results
    x = np.ascontiguousarray(np.asarray(x, dtype=np.float32))
    pseudo_index = np.ascontiguousarray(np.asarray(pseudo_index, dtype=np.float32))
    weight = np.asarray(weight, dtype=np.float32)
    bias = np.asarray(bias, dtype=np.float32)
    lora_A = np.asarray(lora_A, dtype=np.float32)
    lora_B = np.asarray(lora_B, dtype=np.float32)

    nc = _build()
    in_maps = _pack_inputs(x, pseudo_index, weight, bias, lora_A, lora_B)
    res = run_bass_kernel_spmd(nc, in_maps, list(range(NCORES)))
    last_results = res
    return np.hstack(
        [res.results[i]["out"] for i in range(NCORES)]
    ).astype(np.float32)
